# revision 7
# baseline (speedup 1.0000x reference)
"""DiffPool pooling layer on 8 Trainium2 NeuronCores.

Reference computation (edge_index / batch are unused by the output):
    s      = softmax(x @ Wp + bp, axis=-1)        # [N, C]
    h      = x @ We + be                          # [N, F]
    pooled = s^T @ h                              # [C, F]
    out    = pooled[None] @ Wo + bo               # [1, C, O]

Algebraic restructuring (everything after s is linear):
    out = (s^T x) (We Wo) + colsum(s) (be Wo) + bo
so per node-shard k each core computes only
    G_k  = s_k^T x_k            [C, F]   (PSUM accumulated)
    cs_k = colsum(s_k)          [C, 1]
    out_k = G_k @ W2            [C, O]   with W2 = We Wo (host-precomputed)
and the host computes sum_k out_k + outer(sum_k cs_k, be Wo) + bo.

Perf notes (vs the earlier 69.5us version):
  - x is cast fp32->fp16 on the HOST, so the device reads 6.4MB instead
    of 12.8MB, via HWDGE (SP engine) instead of SWDGE cast-DMA (the
    SWDGE cast path alone measured 71.6us; HWDGE fp32 was 49us).
  - W2 host-fusion removes the We (1MB) load and one matmul chain.
  - cs computed as [C, 1] (1 PE row/tile instead of 64).
  - softmax normalize on the (otherwise idle) Pool engine; exp writes
    fp16 directly.
  - w2 is DMA'd after the x stream on the SP queue (needed only at the
    final projection); wp/bp (tiny) go first.

Layout: nodes are block-assigned to partitions (partition p holds nodes
p*48..p*48+47 of the first 6144; the 106-node tail is node-major). Any
node->partition assignment is valid because the G contraction only
requires s and x to agree on it.

Per 128-node tile j (x resident in SBUF as fp16):
  - PE transposes 4 f-chunks -> xT (fp16 PSUM) -> DVE copy to SBUF
  - logits = ones x bp + sum_k xT_k^T @ Wp_k    (fp16 MMs, fp32 PSUM)
  - ACT exp -> fp16 unnormalized s + fp32 row sums; DVE recip;
    Pool scale -> s
  - cs/G matmuls are software-pipelined SKEW tiles behind; G's 512-row
    stream is emitted last so following stationary loads hide under it.
Final: out = G @ W2 via 4 fp32r matmuls (1 cyc/row at free=256).
"""

import numpy as np
from contextlib import ExitStack

N_ALL, F, C, O = 50000, 512, 64, 256
NCORES = 8
NLOC = N_ALL // NCORES          # 6250 nodes per core
P = 128
KC = F // P                     # 4 feature chunks
JROWS = 48                      # node tiles in the main block
NMAIN = P * JROWS               # 6144 nodes in the main block
NTAIL = NLOC - NMAIN            # 106-node tail
NSPLIT = 12                     # main-block DMA slices (4 tiles each)
JS = JROWS // NSPLIT            # tiles per slice

_CACHE = {}


def _main_loop(nc, mybir, x_d, w2_d, xs_parts, x_tail, w2_sb,
               xtpool, spool, smallp, pxt, plg,
               ident16, ones_row16, ones_col16, wp_h, bp_h, g_ps, cs_ps,
               parts="full"):
    """One full pass over this core's node shard, accumulating G / colsum."""
    f32 = mybir.dt.float32
    f16 = mybir.dt.float16
    AF = mybir.ActivationFunctionType

    # x DMAs: HWDGE (SP queue), fp16, 4KB-contiguous per partition line
    xm = x_d[0:NMAIN, :].rearrange("(p j) f -> p j f", p=P)
    if parts != "nodma":
        for i in range(NSPLIT):
            nc.sync.dma_start(xs_parts[i][:], xm[:, i * JS : (i + 1) * JS, :])
        nc.sync.dma_start(x_tail[0:NTAIL, :], x_d[NMAIN:NLOC, :])
    if w2_d is not None:
        # after the x stream on the same SP queue: only needed at the end
        nc.sync.dma_start(w2_sb[:], w2_d)

    if parts == "dma":
        return

    # tile list: (x view full-partition, active rows)
    tiles = [(xs_parts[j // JS][:, j % JS, :], P) for j in range(JROWS)]
    tiles.append((x_tail[:, :], NTAIL))
    ntiles = len(tiles)

    # software pipeline so PE never waits on DVE/ACT/Pool:
    # at step j, PE runs: transp(j+1) | logits(j) | cs/G(j-SKEW)
    xt_sbs = {}   # j -> xt_sb
    s_views = {}  # j -> s view for cs/G

    def emit_transp(j):
        xv, nt = tiles[j]
        xt_ps = pxt.tile([P, KC, P], f16, tag="xt_ps", name="xt_ps")
        for k in range(KC):
            nc.tensor.transpose(
                xt_ps[:, k, 0:nt],
                xv[0:nt, k * P : (k + 1) * P],
                ident16[0:nt, 0:nt],
            )
        xt_sb = xtpool.tile([P, KC, P], f16, tag="xt_sb", name="xt_sb")
        # fp16 pairs copied as fp32 halves the DVE element count
        nc.vector.tensor_copy(
            xt_sb[:, :, 0:nt].bitcast(f32), xt_ps[:, :, 0:nt].bitcast(f32)
        )
        xt_sbs[j] = xt_sb

    def emit_logits(j):
        _, nt = tiles[j]
        xt_sb = xt_sbs.pop(j)
        lg_ps = plg.tile([P, C], f32, tag="lg_ps", name="lg_ps")
        nc.tensor.matmul(
            lg_ps[0:nt, :], ones_row16[:, 0:nt], bp_h[:],
            start=True, stop=False,
        )
        for k in range(KC):
            nc.tensor.matmul(
                lg_ps[0:nt, :], xt_sb[:, k, 0:nt], wp_h[:, k, :],
                start=False, stop=(k == KC - 1),
            )
        return lg_ps

    def emit_softmax(j, lg_ps):
        _, nt = tiles[j]
        if parts == "nosm":
            s_views[j] = ident16[0:nt, 0:C]
            return
        se_h = spool.tile([P, C], f16, tag="se_h", name="se_h")
        rs = smallp.tile([P, 1], f32, tag="rs", name="rs")
        nc.scalar.activation(
            se_h[0:nt, :], lg_ps[0:nt, :], AF.Exp, accum_out=rs[0:nt, :]
        )
        ri = smallp.tile([P, 1], f32, tag="ri", name="ri")
        nc.vector.reciprocal(ri[0:nt, :], rs[0:nt, :])
        s_h = spool.tile([P, C], f16, tag="s_h", name="s_h")
        nc.gpsimd.tensor_scalar_mul(s_h[0:nt, :], se_h[0:nt, :], ri[0:nt, :])
        s_views[j] = s_h[0:nt, :]

    def emit_gcs(j, last):
        xv, nt = tiles[j]
        s_view = s_views.pop(j)
        # cs first (1-row stream); G's 512-row stream last so the next
        # instructions' stationary loads hide under it
        nc.tensor.matmul(
            cs_ps[:], s_view, ones_col16[0:nt, :],
            start=(j == 0), stop=last,
        )
        nc.tensor.matmul(
            g_ps[:], s_view, xv[0:nt, :],
            start=(j == 0), stop=last,
        )

    SKEW = 4
    emit_transp(0)
    for j in range(ntiles):
        if j + 1 < ntiles:
            emit_transp(j + 1)
        lg_ps = emit_logits(j)
        if j >= SKEW:
            emit_gcs(j - SKEW, last=False)
        emit_softmax(j, lg_ps)
    for j in range(ntiles - SKEW, ntiles):
        emit_gcs(j, last=(j == ntiles - 1))


def _final(nc, mybir, fin, pfin, ident32, w2_sb, g_ps, cs_ps, out_d, cs_d):
    """Project the per-core partial: out = G @ W2 (fp16), export cs."""
    f32 = mybir.dt.float32
    f16 = mybir.dt.float16

    g_sb = fin.tile([C, F], f32, tag="g_sb", name="g_sb")
    nc.vector.tensor_copy(g_sb[:], g_ps[:])
    cs_sb = fin.tile([C, 1], f32, tag="cs_sb", name="cs_sb")
    nc.vector.tensor_copy(cs_sb[:], cs_ps[:])
    nc.sync.dma_start(cs_d, cs_sb[:])

    # G^T chunks [128, C] so f lands on partitions
    gt_ps = pfin.tile([P, KC, C], f32, tag="gt_ps", name="gt_ps")
    for k in range(KC):
        nc.tensor.transpose(
            gt_ps[:, k, :], g_sb[:, k * P : (k + 1) * P], ident32[:]
        )
    gt_sb = fin.tile([P, KC, C], f16, tag="gt_sb", name="gt_sb")
    nc.vector.tensor_copy(gt_sb[:], gt_ps[:])

    # out[c, o] = sum_f G^T[f, c] W2[f, o]   (fp16: 1 cyc/row)
    out_ps = pfin.tile([C, O], f32, tag="out_ps", name="out_ps")
    for k in range(KC):
        nc.tensor.matmul(
            out_ps[:], gt_sb[:, k, :], w2_sb[:, k, :],
            start=(k == 0), stop=(k == KC - 1),
        )
    out_sb = fin.tile([C, O], f32, tag="out_sb", name="out_sb")
    nc.vector.tensor_copy(out_sb[:], out_ps[:])
    nc.sync.dma_start(out_d, out_sb[:])


def _build(bench_reps=None, parts="full"):
    """Build the bass module. bench_reps: if set, wrap the whole pass
    (x DMA + compute + final projection) in a hardware For_i repeating it
    that many times (timing-only variant: x and weights live on device,
    no input transfer)."""
    import concourse.mybir as mybir
    import concourse.tile as tile
    from concourse import bacc
    from concourse.masks import make_identity

    f32 = mybir.dt.float32
    f16 = mybir.dt.float16

    nc = bacc.Bacc(
        "TRN2", target_bir_lowering=False, debug=False, num_devices=NCORES
    )

    if bench_reps:
        x_d = nc.dram_tensor("xint", [NLOC, F], f16, kind="Internal").ap()
        wp_d = bp_d = w2_d = None
    else:
        x_d = nc.dram_tensor("x", [NLOC, F], f16, kind="ExternalInput").ap()
        wp_d = nc.dram_tensor("wp", [P, KC, C], f16, kind="ExternalInput").ap()
        bp_d = nc.dram_tensor("bp", [1, C], f16, kind="ExternalInput").ap()
        w2_d = nc.dram_tensor("w2", [P, KC, O], f16, kind="ExternalInput").ap()
    out_d = nc.dram_tensor("out", [C, O], f32, kind="ExternalOutput").ap()
    cs_d = nc.dram_tensor("cs", [C, 1], f32, kind="ExternalOutput").ap()

    with tile.TileContext(nc) as tc, ExitStack() as ctx:
        const = ctx.enter_context(tc.tile_pool(name="const", bufs=1))
        accp = ctx.enter_context(tc.tile_pool(name="accp", bufs=1, space="PSUM"))

        ident16 = const.tile([P, P], f16)
        make_identity(nc, ident16[:])
        ident32 = const.tile([C, C], f32)
        make_identity(nc, ident32[:])
        ones_row16 = const.tile([1, P], f16)
        nc.vector.memset(ones_row16[:], 1.0)
        ones_col16 = const.tile([P, 1], f16)
        nc.vector.memset(ones_col16[:], 1.0)

        # resident x (fp16): NSPLIT main slices + node-major tail
        xs_parts = [
            const.tile([P, JS, F], f16, name=f"xs{i}") for i in range(NSPLIT)
        ]
        x_tail = const.tile([P, F], f16, name="x_tail")

        # weights (host pre-arranged: [P, KC, M], partition = f within chunk)
        wp_h = const.tile([P, KC, C], f16)
        bp_h = const.tile([1, C], f16)
        w2_sb = const.tile([P, KC, O], f16)
        if bench_reps:
            for tl in (wp_h, bp_h, w2_sb):
                nc.vector.memset(tl[:], 0.0)
        else:
            nc.sync.dma_start(wp_h[:], wp_d)
            nc.sync.dma_start(bp_h[:], bp_d)
            # w2 DMA is issued inside _main_loop, after the x stream

        # persistent accumulators
        g_ps = accp.tile([C, F], f32)
        cs_ps = accp.tile([C, 1], f32)

        if parts == "nodma":
            # ablation: compute-only — fill resident x once, no per-pass DMA
            for t in xs_parts:
                nc.vector.memset(t[:], 0.0)
            nc.vector.memset(x_tail[:], 0.0)
        if bench_reps:
            # zero-fill internal x so the compute sees finite data
            zt = const.tile([P, JS, F], f16, name="zt")
            nc.vector.memset(zt[:], 0.0)
            xm = x_d[0:NMAIN, :].rearrange("(p j) f -> p j f", p=P)
            for i in range(NSPLIT):
                nc.sync.dma_start(xm[:, i * JS : (i + 1) * JS, :], zt[:])
            nc.sync.dma_start(x_d[NMAIN:NLOC, :], zt[0:NTAIL, 0, :])

        with ExitStack() as lctx:
            xtpool = lctx.enter_context(tc.tile_pool(name="xtpool", bufs=4))
            spool = lctx.enter_context(tc.tile_pool(name="spool", bufs=10))
            smallp = lctx.enter_context(tc.tile_pool(name="smallp", bufs=10))
            pxt = lctx.enter_context(
                tc.tile_pool(name="pxt", bufs=2, space="PSUM")
            )
            plg = lctx.enter_context(
                tc.tile_pool(name="plg", bufs=2, space="PSUM")
            )
            fin = lctx.enter_context(tc.tile_pool(name="fin", bufs=1))
            pfin = lctx.enter_context(
                tc.tile_pool(name="pfin", bufs=1, space="PSUM")
            )

            rep_ctx = (
                tc.For_i(0, bench_reps, 1) if bench_reps else ExitStack()
            )
            with rep_ctx:
                _main_loop(
                    nc, mybir, x_d, w2_d, xs_parts, x_tail, w2_sb,
                    xtpool, spool, smallp, pxt, plg,
                    ident16, ones_row16, ones_col16, wp_h, bp_h,
                    g_ps, cs_ps, parts=parts,
                )
                if parts in ("dma", "nodma"):
                    dummy = fin.tile([C, O], f32, tag="dummy", name="dummy")
                    nc.vector.memset(dummy[:], 0.0)
                    nc.sync.dma_start(out_d, dummy[:])
                    nc.sync.dma_start(cs_d, dummy[:, 0:1])
                else:
                    _final(nc, mybir, fin, pfin, ident32, w2_sb,
                           g_ps, cs_ps, out_d, cs_d)

    nc.compile()
    return nc


def _get_nc(bench_reps=None, parts="full"):
    key = ("nc", bench_reps, parts)
    if key not in _CACHE:
        _CACHE[key] = _build(bench_reps, parts)
    return _CACHE[key]


def kernel(x, edge_index=None, batch=None, Wp=None, bp=None, We=None,
           be=None, Wo=None, bo=None, **_unused):
    from concourse.bass_utils import run_bass_kernel_spmd

    x16 = np.ascontiguousarray(
        np.asarray(x, dtype=np.float32).astype(np.float16)
    )
    wp16 = np.ascontiguousarray(
        np.asarray(Wp, dtype=np.float32).astype(np.float16)
        .reshape(KC, P, C).transpose(1, 0, 2)
    )
    bp16 = np.ascontiguousarray(
        np.asarray(bp, dtype=np.float32).astype(np.float16).reshape(1, C)
    )
    We_ = np.asarray(We, dtype=np.float32)
    Wo_ = np.asarray(Wo, dtype=np.float32)
    w2 = np.ascontiguousarray(
        (We_ @ Wo_).astype(np.float16).reshape(KC, P, O).transpose(1, 0, 2)
    )
    bewo = (np.asarray(be, dtype=np.float32).reshape(1, F) @ Wo_)[0]  # [O]
    bo_ = np.asarray(bo, dtype=np.float32).reshape(O)

    nc = _get_nc()
    in_maps = []
    for k in range(NCORES):
        in_maps.append(
            {
                "x": np.ascontiguousarray(x16[k * NLOC : (k + 1) * NLOC]),
                "wp": wp16,
                "bp": bp16,
                "w2": w2,
            }
        )
    res = run_bass_kernel_spmd(nc, in_maps, core_ids=list(range(NCORES)))
    out = np.zeros((C, O), np.float64)
    cs = np.zeros((C,), np.float64)
    for r in res.results:
        out += r["out"]
        cs += r["cs"][:, 0]
    out = out + np.outer(cs, bewo) + bo_[None, :]
    return out[None].astype(np.float32)  # [1, C, O]


# revision 19
# speedup vs baseline: 1.5528x; 1.5528x over previous
"""DiffPool pooling layer on 8 Trainium2 NeuronCores.

Reference computation (edge_index / batch are unused by the output):
    s      = softmax(x @ Wp + bp, axis=-1)        # [N, C]
    h      = x @ We + be                          # [N, F]
    pooled = s^T @ h                              # [C, F]
    out    = pooled[None] @ Wo + bo               # [1, C, O]

Algebraic restructuring (everything after s is linear):
    out = (s^T x) (We Wo) + colsum(s) (be Wo) + bo
so per node-shard k each core computes only
    G_k  = s_k^T x_k            [C, F]   (PSUM accumulated)
    cs_k = colsum(s_k)          [C, 1]
    out_k = G_k @ W2            [C, O]   with W2 = We Wo (host-precomputed)
and the host computes sum_k out_k + outer(sum_k cs_k, be Wo) + bo.

Perf notes (vs the earlier 69.5us version):
  - x is cast fp32->fp16 on the HOST, so the device reads 6.4MB instead
    of 12.8MB, via HWDGE (SP engine) instead of SWDGE cast-DMA (the
    SWDGE cast path alone measured 71.6us; HWDGE fp32 was 49us).
  - W2 host-fusion removes the We (1MB) load and one matmul chain.
  - cs computed as [C, 1] (1 PE row/tile instead of 64).
  - softmax normalize on the (otherwise idle) Pool engine; exp writes
    fp16 directly.
  - w2 is DMA'd after the x stream on the SP queue (needed only at the
    final projection); wp/bp (tiny) go first.

Layout: nodes are block-assigned to partitions (partition p holds nodes
p*48..p*48+47 of the first 6144; the 106-node tail is node-major). Any
node->partition assignment is valid because the G contraction only
requires s and x to agree on it.

Per 128-node tile j (x resident in SBUF as fp16):
  - PE transposes 4 f-chunks -> xT (fp16 PSUM) -> DVE copy to SBUF
  - logits = ones x bp + sum_k xT_k^T @ Wp_k    (fp16 MMs, fp32 PSUM)
  - ACT exp -> fp16 unnormalized s + fp32 row sums; DVE recip;
    Pool scale -> s
  - cs/G matmuls are software-pipelined SKEW tiles behind; G's 512-row
    stream is emitted last so following stationary loads hide under it.
Final: out = G @ W2 via 4 fp32r matmuls (1 cyc/row at free=256).
"""

import numpy as np
from contextlib import ExitStack

N_ALL, F, C, O = 50000, 512, 64, 256
NCORES = 8
NLOC = N_ALL // NCORES          # 6250 nodes per core
P = 128
KC = F // P                     # 4 feature chunks
JROWS = 48                      # node tiles in the main block
NMAIN = P * JROWS               # 6144 nodes in the main block
NTAIL = NLOC - NMAIN            # 106-node tail
NSPLIT = 12                     # main-block DMA slices (4 tiles each)
JS = JROWS // NSPLIT            # tiles per slice

_CACHE = {}


def _nsplit_of(flags):
    if "ns6" in flags:
        return 6
    if "ns8" in flags:
        return 8
    return NSPLIT


def _main_loop(nc, mybir, x_d, w2_d, xs_parts, x_tail, w2_sb,
               xtpool, spool, smallp, pxt, plg,
               ident16, ones_row16, ones_col16, wp_h, bp_h, bias128,
               g_ps, cs_ps, parts="full", xs_scratch=None):
    """One full pass over this core's node shard, accumulating G / colsum."""
    f32 = mybir.dt.float32
    f16 = mybir.dt.float16
    AF = mybir.ActivationFunctionType
    flags = set(parts.split(","))
    nsplit = _nsplit_of(flags)
    js = JROWS // nsplit

    # x DMAs: HWDGE (SP queue), fp16, js-KB-contiguous per partition line
    xm = x_d[0:NMAIN, :].rearrange("(p j) f -> p j f", p=P)
    if "dmabg" in flags:
        # contention probe: stream x into dead scratch, compute on resident x
        for i in range(nsplit):
            nc.sync.dma_start(
                xs_scratch[i][:], xm[:, i * js : (i + 1) * js, :])
    elif "nodma" not in flags:
        # tail first: it is tiny, so compute can start almost immediately
        nc.sync.dma_start(x_tail[0:NTAIL, :], x_d[NMAIN:NLOC, :])
        for i in range(nsplit):
            nc.sync.dma_start(xs_parts[i][:], xm[:, i * js : (i + 1) * js, :])
    if w2_d is not None:
        # ACT HWDGE queue: keeps the SP queue a pure x stream
        nc.scalar.dma_start(w2_sb[:], w2_d)

    if "dma" in flags:
        return

    # tile list: (x view full-partition, active rows); tail first to
    # match the DMA order above
    tiles = [(x_tail[:, :], NTAIL)]
    tiles += [(xs_parts[j // js][:, j % js, :], P) for j in range(JROWS)]
    ntiles = len(tiles)

    # software pipeline so PE never waits on DVE/ACT/Pool:
    # at step j, PE runs: transp(j+1) | logits(j) | cs/G(j-SKEW)
    xt_sbs = {}   # j -> xt_sb
    s_views = {}  # j -> s view for cs/G

    def emit_transp(j):
        xv, nt = tiles[j]
        xt_ps = pxt.tile([P, KC, P], f16, tag="xt_ps", name="xt_ps")
        for k in range(KC):
            nc.tensor.transpose(
                xt_ps[:, k, 0:nt],
                xv[0:nt, k * P : (k + 1) * P],
                ident16[0:nt, 0:nt],
            )
        xt_sb = xtpool.tile([P, KC, P], f16, tag="xt_sb", name="xt_sb")
        # fp16 pairs copied as fp32 halves the element count; alternate
        # engines so neither DVE nor ACT becomes the serial resource
        if "actcopy" in flags and (j % 2 == 1):
            nc.scalar.activation(
                xt_sb[:, :, 0:nt].bitcast(f32), xt_ps[:, :, 0:nt].bitcast(f32),
                AF.Copy,
            )
        else:
            nc.vector.tensor_copy(
                xt_sb[:, :, 0:nt].bitcast(f32), xt_ps[:, :, 0:nt].bitcast(f32)
            )
        xt_sbs[j] = xt_sb

    def emit_logits(j):
        _, nt = tiles[j]
        xt_sb = xt_sbs.pop(j)
        lg_ps = plg.tile([P, C], f32, tag="lg_ps", name="lg_ps")
        if "pebias" in flags:
            nc.tensor.matmul(
                lg_ps[0:nt, :], ones_row16[:, 0:nt], bp_h[:],
                start=True, stop=False,
            )
        else:
            # init the PSUM accumulator with the broadcast bias from DVE;
            # the chunk matmuls then accumulate on top (start=False)
            nc.vector.tensor_copy(lg_ps[0:nt, :], bias128[0:nt, :])
        for k in range(KC):
            nc.tensor.matmul(
                lg_ps[0:nt, :], xt_sb[:, k, 0:nt], wp_h[:, k, :],
                start=False, stop=(k == KC - 1),
                skip_group_check=("pebias" not in flags),
            )
        return lg_ps

    def emit_softmax(j, lg_ps):
        _, nt = tiles[j]
        if "nosm" in flags:
            s_views[j] = ident16[0:nt, 0:C]
            return
        se_h = spool.tile([P, C], f16, tag="se_h", name="se_h")
        rs = smallp.tile([P, 1], f32, tag="rs", name="rs")
        nc.scalar.activation(
            se_h[0:nt, :], lg_ps[0:nt, :], AF.Exp, accum_out=rs[0:nt, :]
        )
        s_h = spool.tile([P, C], f16, tag="s_h", name="s_h")
        if "divnorm" in flags:
            # one DVE op: s = se / rowsum  (skips the separate reciprocal)
            nc.vector.tensor_scalar(
                s_h[0:nt, :], se_h[0:nt, :], rs[0:nt, :], None,
                mybir.AluOpType.divide,
            )
        else:
            ri = smallp.tile([P, 1], f32, tag="ri", name="ri")
            nc.vector.reciprocal(ri[0:nt, :], rs[0:nt, :])
            if "poolmul" in flags:
                nc.gpsimd.tensor_scalar_mul(
                    s_h[0:nt, :], se_h[0:nt, :], ri[0:nt, :])
            else:
                nc.vector.tensor_scalar_mul(
                    s_h[0:nt, :], se_h[0:nt, :], ri[0:nt, :])
        s_views[j] = s_h[0:nt, :]

    def emit_gcs(j, last):
        xv, nt = tiles[j]
        s_view = s_views.pop(j)
        if "csold" in flags:
            nc.tensor.matmul(
                g_ps[:], s_view, xv[0:nt, :],
                start=(j == 0), stop=last,
            )
            nc.tensor.matmul(
                cs_ps[:], ones_col16[0:nt, :], s_view,
                start=(j == 0), stop=last,
            )
        else:
            # cs first (1-row stream); G's 512-row stream last so the next
            # instructions' stationary loads hide under it
            nc.tensor.matmul(
                cs_ps[:], s_view, ones_col16[0:nt, :],
                start=(j == 0), stop=last,
            )
            nc.tensor.matmul(
                g_ps[:], s_view, xv[0:nt, :],
                start=(j == 0), stop=last,
            )

    do_logits = not ({"transp", "nologits"} & flags)
    do_gcs = not ({"nogcs", "transp"} & flags)
    if {"transp", "nologits"} & flags:
        for j in range(ntiles):
            s_views[j] = ident16[0 : tiles[j][1], 0:C]

    SKEW = 4 if "sk4" in flags else (8 if "sk8" in flags else 6)
    emit_transp(0)
    for j in range(ntiles):
        if j + 1 < ntiles:
            emit_transp(j + 1)
        if do_logits:
            lg_ps = emit_logits(j)
        else:
            xt_sbs.pop(j)
        if do_gcs and j >= SKEW:
            emit_gcs(j - SKEW, last=False)
        if do_logits:
            emit_softmax(j, lg_ps)
    if do_gcs:
        for j in range(ntiles - SKEW, ntiles):
            emit_gcs(j, last=(j == ntiles - 1))


def _final(nc, mybir, fin, pxt, plg, ident32, w2_sb, g_ps, cs_ps, out_d, cs_d):
    """Project the per-core partial: out = G @ W2 (fp16), export cs.

    PSUM scratch reuses the main-loop rings (pxt / plg) so the loop can
    keep 3 bufs each within the 8-bank budget.
    """
    f32 = mybir.dt.float32
    f16 = mybir.dt.float16

    g_sb = fin.tile([C, F], f32, tag="g_sb", name="g_sb")
    nc.vector.tensor_copy(g_sb[:], g_ps[:])
    cs_sb = fin.tile([C, 1], f32, tag="cs_sb", name="cs_sb")
    nc.vector.tensor_copy(cs_sb[:], cs_ps[:])
    nc.scalar.dma_start(cs_d, cs_sb[:])

    # G^T chunks [128, C] so f lands on partitions
    gt_ps = pxt.tile([P, KC, C], f32, tag="xt_ps", name="gt_ps")
    for k in range(KC):
        nc.tensor.transpose(
            gt_ps[:, k, :], g_sb[:, k * P : (k + 1) * P], ident32[:]
        )
    gt_sb = fin.tile([P, KC, C], f16, tag="gt_sb", name="gt_sb")
    nc.vector.tensor_copy(gt_sb[:], gt_ps[:])

    # out[c, o] = sum_f G^T[f, c] W2[f, o]   (fp16: 1 cyc/row)
    out_ps = plg.tile([C, O], f32, tag="lg_ps", name="out_ps")
    for k in range(KC):
        nc.tensor.matmul(
            out_ps[:], gt_sb[:, k, :], w2_sb[:, k, :],
            start=(k == 0), stop=(k == KC - 1),
        )
    out_sb = fin.tile([C, O], f32, tag="out_sb", name="out_sb")
    nc.vector.tensor_copy(out_sb[:], out_ps[:])
    nc.scalar.dma_start(out_d, out_sb[:])


def _build(bench_reps=None, parts="full"):
    """Build the bass module. bench_reps: if set, wrap the whole pass
    (x DMA + compute + final projection) in a hardware For_i repeating it
    that many times (timing-only variant: x and weights live on device,
    no input transfer)."""
    import concourse.mybir as mybir
    import concourse.tile as tile
    from concourse import bacc
    from concourse.masks import make_identity

    f32 = mybir.dt.float32
    f16 = mybir.dt.float16

    nc = bacc.Bacc(
        "TRN2", target_bir_lowering=False, debug=False, num_devices=NCORES
    )

    if bench_reps:
        x_d = nc.dram_tensor("xint", [NLOC, F], f16, kind="Internal").ap()
        wp_d = bp_d = w2_d = None
    else:
        x_d = nc.dram_tensor("x", [NLOC, F], f16, kind="ExternalInput").ap()
        wp_d = nc.dram_tensor("wp", [P, KC, C], f16, kind="ExternalInput").ap()
        bp_d = nc.dram_tensor("bp", [1, C], f16, kind="ExternalInput").ap()
        w2_d = nc.dram_tensor("w2", [P, KC, O], f16, kind="ExternalInput").ap()
    out_d = nc.dram_tensor("out", [C, O], f32, kind="ExternalOutput").ap()
    cs_d = nc.dram_tensor("cs", [C, 1], f32, kind="ExternalOutput").ap()

    with tile.TileContext(nc) as tc, ExitStack() as ctx:
        const = ctx.enter_context(tc.tile_pool(name="const", bufs=1))
        accp = ctx.enter_context(tc.tile_pool(name="accp", bufs=1, space="PSUM"))

        ident16 = const.tile([P, P], f16)
        make_identity(nc, ident16[:])
        ident32 = const.tile([C, C], f32)
        make_identity(nc, ident32[:])
        ones_row16 = const.tile([1, P], f16)
        nc.vector.memset(ones_row16[:], 1.0)
        ones_col16 = const.tile([P, 1], f16)
        nc.vector.memset(ones_col16[:], 1.0)

        # resident x (fp16): nsplit main slices + node-major tail
        nsplit = _nsplit_of(set(parts.split(",")))
        js = JROWS // nsplit
        xs_parts = [
            const.tile([P, js, F], f16, name=f"xs{i}") for i in range(nsplit)
        ]
        x_tail = const.tile([P, F], f16, name="x_tail")

        # weights (host pre-arranged: [P, KC, M], partition = f within chunk)
        wp_h = const.tile([P, KC, C], f16)
        bp_h = const.tile([1, C], f16)
        w2_sb = const.tile([P, KC, O], f16)
        if bench_reps:
            for tl in (wp_h, bp_h, w2_sb):
                nc.vector.memset(tl[:], 0.0)
        else:
            nc.sync.dma_start(wp_h[:], wp_d)
            nc.sync.dma_start(bp_h[:], bp_d)
            # w2 DMA is issued inside _main_loop, after the x stream

        flags = set(parts.split(","))
        # broadcast bias [P, C] (setup-only cost)
        bias128 = const.tile([P, C], f32)
        # persistent accumulators
        g_ps = accp.tile([C, F], f32)
        cs_ps = accp.tile([1, C] if "csold" in flags else [C, 1], f32)

        xs_scratch = None
        if "dmabg" in flags:
            xs_scratch = [
                const.tile([P, js, F], f16, name=f"xscr{i}")
                for i in range(nsplit)
            ]
        with tc.tile_pool(name="pinit", bufs=1, space="PSUM") as pinit:
            b_ps = pinit.tile([P, C], f32, name="b_ps")
            nc.tensor.matmul(
                b_ps[:], ones_row16[:], bp_h[:], start=True, stop=True
            )
            nc.vector.tensor_copy(bias128[:], b_ps[:])
        if "nodma" in flags:
            # ablation: compute-only — fill resident x once, no per-pass DMA
            for t in xs_parts:
                nc.vector.memset(t[:], 0.0)
            nc.vector.memset(x_tail[:], 0.0)
        if bench_reps:
            # zero-fill internal x so the compute sees finite data
            zt = const.tile([P, js, F], f16, name="zt")
            nc.vector.memset(zt[:], 0.0)
            xm = x_d[0:NMAIN, :].rearrange("(p j) f -> p j f", p=P)
            for i in range(nsplit):
                nc.sync.dma_start(xm[:, i * js : (i + 1) * js, :], zt[:])
            nc.sync.dma_start(x_d[NMAIN:NLOC, :], zt[0:NTAIL, 0, :])

        with ExitStack() as lctx:
            xtpool = lctx.enter_context(tc.tile_pool(name="xtpool", bufs=4))
            spool = lctx.enter_context(tc.tile_pool(name="spool", bufs=10))
            smallp = lctx.enter_context(tc.tile_pool(name="smallp", bufs=10))
            pxt = lctx.enter_context(
                tc.tile_pool(name="pxt", bufs=3, space="PSUM")
            )
            plg = lctx.enter_context(
                tc.tile_pool(name="plg", bufs=3, space="PSUM")
            )
            fin = lctx.enter_context(tc.tile_pool(name="fin", bufs=1))

            rep_ctx = (
                tc.For_i(0, bench_reps, 1) if bench_reps else ExitStack()
            )
            with rep_ctx:
                _main_loop(
                    nc, mybir, x_d, w2_d, xs_parts, x_tail, w2_sb,
                    xtpool, spool, smallp, pxt, plg,
                    ident16, ones_row16, ones_col16, wp_h, bp_h, bias128,
                    g_ps, cs_ps, parts=parts, xs_scratch=xs_scratch,
                )
                if {"dma", "nodma", "transp", "nologits", "nogcs", "nosm",
                    "csold"} & flags:
                    dummy = fin.tile([C, O], f32, tag="dummy", name="dummy")
                    nc.vector.memset(dummy[:], 0.0)
                    nc.scalar.dma_start(out_d, dummy[:])
                    nc.scalar.dma_start(cs_d, dummy[:, 0:1])
                else:
                    _final(nc, mybir, fin, pxt, plg, ident32, w2_sb,
                           g_ps, cs_ps, out_d, cs_d)

    nc.compile()
    return nc


def _get_nc(bench_reps=None, parts="full"):
    key = ("nc", bench_reps, parts)
    if key not in _CACHE:
        _CACHE[key] = _build(bench_reps, parts)
    return _CACHE[key]


def kernel(x, edge_index=None, batch=None, Wp=None, bp=None, We=None,
           be=None, Wo=None, bo=None, **_unused):
    from concourse.bass_utils import run_bass_kernel_spmd

    x16 = np.ascontiguousarray(
        np.asarray(x, dtype=np.float32).astype(np.float16)
    )
    wp16 = np.ascontiguousarray(
        np.asarray(Wp, dtype=np.float32).astype(np.float16)
        .reshape(KC, P, C).transpose(1, 0, 2)
    )
    bp16 = np.ascontiguousarray(
        np.asarray(bp, dtype=np.float32).astype(np.float16).reshape(1, C)
    )
    We_ = np.asarray(We, dtype=np.float32)
    Wo_ = np.asarray(Wo, dtype=np.float32)
    w2 = np.ascontiguousarray(
        (We_ @ Wo_).astype(np.float16).reshape(KC, P, O).transpose(1, 0, 2)
    )
    bewo = (np.asarray(be, dtype=np.float32).reshape(1, F) @ Wo_)[0]  # [O]
    bo_ = np.asarray(bo, dtype=np.float32).reshape(O)

    nc = _get_nc()
    in_maps = []
    for k in range(NCORES):
        in_maps.append(
            {
                "x": np.ascontiguousarray(x16[k * NLOC : (k + 1) * NLOC]),
                "wp": wp16,
                "bp": bp16,
                "w2": w2,
            }
        )
    res = run_bass_kernel_spmd(nc, in_maps, core_ids=list(range(NCORES)))
    out = np.zeros((C, O), np.float64)
    cs = np.zeros((C,), np.float64)
    for r in res.results:
        out += r["out"]
        cs += r["cs"][:, 0]
    out = out + np.outer(cs, bewo) + bo_[None, :]
    return out[None].astype(np.float32)  # [1, C, O]
